# revision 1
# baseline (speedup 1.0000x reference)
"""Trainium2 Bass kernel for CustomAttention (ViT-style windowed attention).

Math (per batch element):
  qkv = x @ qkv_w.T + qkv_b            -> q, k, v  [H=12 heads, D=64]
  s   = (q * D^-0.5) @ k.T             masked by a fixed 24x24-grid window
  attn = softmax(s)                    (CLS row/col always attended)
  out  = attn @ v                      -> concat heads -> @ proj_w.T + proj_b

Sharding: data-parallel over batch across 8 cores (4 images/core).

Device-side layout choices (per core):
  - x fed pre-transposed from host as xT [C, T] so every matmul contracts on
    the partition dim without on-device transposes.
  - all matmul operands are float32r (4-byte, 1 PE pass at N>=256 vs 2
    half-rate passes for float32); accumulation stays fp32 in PSUM.
  - q,k produced feature-major (scale folded into q), v produced token-major
    with an interleaved ones column so the attention@v matmul also yields the
    softmax denominators (row 64 of each [65, n] psum block).
  - softmax runs in k-major layout: exp (no max subtraction; |s| <~ 3) then
    multiply by the 0/1 mask (split across DVE and GPSIMD).
  - normalization is deferred: unnormalized outputs are copied to SBUF, the
    12 heads' denominator rows go to DRAM, one batched reciprocal [12, N]
    computes all inverses, which are broadcast back via DRAM-source
    partition-stride-0 DMA and multiplied in place per head.
"""

import numpy as np

import concourse.bass as bass
import concourse.mybir as mybir
from concourse import bacc
from concourse.bass_utils import run_bass_kernel_spmd
from concourse.tile import TileContext

B, N, C = 32, 577, 768
H, D = 12, 64
NCORES = 8
BPC = B // NCORES            # batches per core
T = BPC * N                  # tokens per core
NP = N + 1                   # q/token free dim padded to even (f32r needs even N)
TP = BPC * NP
SCALE = float(D) ** -0.5
F32 = mybir.dt.float32
F32R = mybir.dt.float32r
P = 128

CT = C // P                              # 6 contraction tiles over channels
KT = [(0, 128), (128, 128), (256, 128), (384, 128), (512, 65)]  # key/token tiles
QCH = [(0, 290), (290, 288)]             # q chunks (>=256 keeps f32r at rate 1)
VCH = [(0, 512), (512, 256)]             # v / proj output chunks
AF = mybir.ActivationFunctionType
ALU = mybir.AluOpType


def _build_mask_np():
    img = 24
    p = np.arange(img * img)
    pi, pj = p // img, p % img
    ok = (np.abs(pi[:, None] - pi[None, :]) <= 1) & (
        np.abs(pj[:, None] - pj[None, :]) <= 1
    )
    m = np.zeros((N, N), dtype=bool)
    m[1:, 1:] = ok
    m[0, :] = True
    m[:, 0] = True
    return m


def _bcast_ap(ap1d, parts):
    """1-row AP -> [parts, n] with partition stride 0 (DRAM-source DMA)."""
    return bass.AP(
        tensor=ap1d.tensor, offset=ap1d.offset, ap=[[0, parts]] + list(ap1d.ap)[-1:]
    )


def _build_program(dbg=False):
    nc = bacc.Bacc("TRN2", target_bir_lowering=False, debug=False)
    dbg_t = {}
    if dbg:
        for name, shape, dt in [
            ("dbg_q", [P, NP], F32R),
            ("dbg_k", [P, NP], F32R),
            ("dbg_v", [P, H * (D + 1)], F32R),
            ("dbg_es", [P, NP], F32R),
            ("dbg_oe", [65, 290], F32),
            ("dbg_rb", [P, NP], F32),
            ("dbg_oc", [P, NP], F32R),
        ]:
            dbg_t[name] = nc.dram_tensor(name, shape, dt, kind="ExternalOutput").ap()
    xT = nc.dram_tensor("xT", [C, TP], F32R, kind="ExternalInput").ap()
    wqkT = nc.dram_tensor("wqkT", [C, 2 * C], F32R, kind="ExternalInput").ap()
    wvT = nc.dram_tensor("wvT", [C, C], F32R, kind="ExternalInput").ap()
    wpT = nc.dram_tensor("wpT", [C, C], F32R, kind="ExternalInput").ap()
    bqk = nc.dram_tensor("bqk", [2 * C], F32, kind="ExternalInput").ap()
    bv = nc.dram_tensor("bv", [C], F32, kind="ExternalInput").ap()
    bp = nc.dram_tensor("bp", [C], F32, kind="ExternalInput").ap()
    maskf = nc.dram_tensor("maskf", [N, NP], F32R, kind="ExternalInput").ap()
    ones12 = nc.dram_tensor("ones12", [H], F32R, kind="ExternalInput").ap()
    y = nc.dram_tensor("y", [T, C], F32, kind="ExternalOutput").ap()

    with TileContext(nc) as tc:
        with (
            tc.tile_pool(name="singles", bufs=1) as singles,
            tc.tile_pool(name="xp", bufs=1) as xp,
            tc.tile_pool(name="qkp", bufs=2) as qkp,
            tc.tile_pool(name="vtp", bufs=1) as vtp,
            tc.tile_pool(name="ocp", bufs=1) as ocp,
            tc.tile_pool(name="esp", bufs=9) as esp,
            tc.tile_pool(name="rcp", bufs=3) as rcpp,
            tc.tile_pool(name="ysp", bufs=2) as ysp,
            tc.tile_pool(name="pmm", bufs=2, space="PSUM") as pmm,
            tc.tile_pool(name="psw", bufs=2, space="PSUM") as psw,
            tc.tile_pool(name="poe", bufs=1, space="PSUM") as poe,
            tc.tile_pool(name="drp", bufs=2, space="DRAM") as drp,
        ):
            # ---- persistent loads ----
            wqk_sb = []
            wv_sb = []
            wp_sb = []
            for ct in range(CT):
                t = singles.tile([P, 2 * C], F32R, tag=f"wqk{ct}")
                nc.sync.dma_start(t[:], wqkT[ct * P : (ct + 1) * P, :])
                wqk_sb.append(t)
                t = singles.tile([P, C], F32R, tag=f"wv{ct}")
                nc.sync.dma_start(t[:], wvT[ct * P : (ct + 1) * P, :])
                wv_sb.append(t)
                t = singles.tile([P, C], F32R, tag=f"wp{ct}")
                nc.sync.dma_start(t[:], wpT[ct * P : (ct + 1) * P, :])
                wp_sb.append(t)
            bqk_sb = singles.tile([P, 2 * C // P], F32, tag="bqk")
            nc.sync.dma_start(bqk_sb[:], bqk.rearrange("(o p) -> p o", p=P))
            bv_sb = singles.tile([P, C], F32, tag="bv")
            nc.sync.dma_start(bv_sb[:], _bcast_ap(bv, P))
            bp_sb = singles.tile([P, C], F32, tag="bp")
            nc.sync.dma_start(bp_sb[:], _bcast_ap(bp, P))
            ones_sb = singles.tile([P, H], F32R, tag="ones_sb")
            nc.sync.dma_start(ones_sb[:], _bcast_ap(ones12, P))
            mask_sb = []
            for kt, (k0, ksz) in enumerate(KT):
                t = singles.tile([P, NP], F32R, tag=f"mask{kt}")
                nc.sync.dma_start(t[:ksz, :], maskf[k0 : k0 + ksz, :])
                mask_sb.append(t)

            def emit_xv(b):
                xT_b = []
                for ct in range(CT):
                    t = xp.tile([P, NP], F32R, tag=f"x{ct}", name=f"x{ct}")
                    nc.sync.dma_start(
                        t[:], xT[ct * P : (ct + 1) * P, b * NP : (b + 1) * NP]
                    )
                    xT_b.append(t)
                v_tok = []
                for mt, (m0, msz) in enumerate(KT):
                    vt = vtp.tile([P, H, D + 1], F32R, tag=f"vt{mt}", name=f"vt{mt}")
                    nc.vector.tensor_copy(vt[:, :, D : D + 1], ones_sb[:, :, None])
                    for ci, (c0, csz) in enumerate(VCH):
                        ps = pmm.tile([P, 512], F32, tag="pb", name="ps")
                        for ct in range(CT):
                            nc.tensor.matmul(
                                ps[:msz, :csz],
                                xT_b[ct][:, m0 : m0 + msz],
                                wv_sb[ct][:, c0 : c0 + csz],
                                start=(ct == 0),
                                stop=(ct == CT - 1),
                            )
                        nh = csz // D
                        h0 = c0 // D
                        nc.vector.tensor_tensor(
                            vt[:msz, h0 : h0 + nh, 0:D],
                            ps[:msz, :csz].rearrange("p (h d) -> p h d", d=D),
                            bv_sb[:msz, c0 : c0 + csz].rearrange(
                                "p (h d) -> p h d", d=D
                            ),
                            ALU.add,
                        )
                    v_tok.append(vt)
                if dbg and b == 0:
                    nc.sync.dma_start(
                        dbg_t["dbg_v"],
                        v_tok[0][:].rearrange("p h d -> p (h d)"),
                    )
                return xT_b, v_tok

            def emit_proj(b, oc_sb):
                for mt, (m0, msz) in enumerate(KT):
                    ysb = ysp.tile([P, C], F32, tag="ysb", name="ysb")
                    for ci, (c0, csz) in enumerate(VCH):
                        ps = pmm.tile([P, 512], F32, tag="pb", name="ps")
                        for ct in range(CT):
                            nc.tensor.matmul(
                                ps[:msz, :csz],
                                oc_sb[ct][:, m0 : m0 + msz],
                                wp_sb[ct][:, c0 : c0 + csz],
                                start=(ct == 0),
                                stop=(ct == CT - 1),
                            )
                        nc.vector.tensor_tensor(
                            ysb[:msz, c0 : c0 + csz],
                            ps[:msz, :csz],
                            bp_sb[:msz, c0 : c0 + csz],
                            ALU.add,
                        )
                    nc.sync.dma_start(
                        y[b * N + m0 : b * N + m0 + msz, :], ysb[:msz, :]
                    )

            xT_b, v_tok = emit_xv(0)
            for b in range(BPC):
                                # ---- per head-pair: qk stage then attention ----
                oc_sb = [
                    ocp.tile([P, NP], F32R, tag=f"oc{ct}", name=f"oc{ct}")
                    for ct in range(CT)
                ]
                srd = drp.tile([H, NP], F32, tag="srd")  # per-head denom rows
                for hp in range(H // 2):
                    # feature-major q (ftile hp, scaled) and k (ftile 6+hp)
                    qt = qkp.tile([P, NP], F32R, tag="qk_q")
                    kt_sb = qkp.tile([P, NP], F32R, tag="qk_k")
                    for dst, ft, scale in ((qt, hp, SCALE), (kt_sb, CT + hp, 1.0)):
                        pss = []
                        for ci, (c0, csz) in enumerate(QCH):
                            ps = pmm.tile([P, 512], F32, tag="pb", name="ps")
                            pss.append(ps)
                            for ct in range(CT):
                                nc.tensor.matmul(
                                    ps[:, :csz],
                                    wqk_sb[ct][:, ft * P : (ft + 1) * P],
                                    xT_b[ct][:, c0 : c0 + csz],
                                    start=(ct == 0),
                                    stop=(ct == CT - 1),
                                )
                        for ci, (c0, csz) in enumerate(QCH):
                            nc.vector.scalar_tensor_tensor(
                                dst[:, c0 : c0 + csz],
                                pss[ci][:, :csz],
                                scale,
                                bqk_sb[:, ft : ft + 1].to_broadcast([P, csz]),
                                ALU.mult,
                                ALU.add,
                            )

                    if dbg and b == 0 and hp == 0:
                        nc.sync.dma_start(dbg_t["dbg_q"], qt[:])
                        nc.sync.dma_start(dbg_t["dbg_k"], kt_sb[:])
                    # both heads of the pair: scores run concurrently on
                    # disjoint PE row groups (K=64 at partition bases 0 / 64)
                    es_t = {0: [], 1: []}
                    for kt, (k0, ksz) in enumerate(KT):
                        for hi, h in enumerate((2 * hp, 2 * hp + 1)):
                            po = 64 * hi
                            es = esp.tile([P, NP + 2], F32R, tag="es", name="es")
                            es_t[hi].append(es)
                            # block (kt=1, chunk 1) is fully masked (window band
                            # is inside chunk 0 and the CLS row/col is not in
                            # this block) -- skip it everywhere
                            chunks = QCH[:1] if kt == 1 else QCH
                            sc = psw.tile([P, 1024], F32, tag="scw", name="sc")
                            for c0, csz in chunks:
                                # chunk 1 goes at column 512 so each matmul
                                # output stays inside one PSUM bank
                                sb0 = 512 if c0 else 0
                                nc.tensor.matmul(
                                    sc[:ksz, sb0 : sb0 + csz],
                                    kt_sb[po : po + D, k0 : k0 + ksz],
                                    qt[po : po + D, c0 : c0 + csz],
                                    start=True,
                                    stop=True,
                                )
                            mw = QCH[0][1] if kt == 1 else NP
                            if kt == 1:
                                nc.scalar.activation(
                                    es[:ksz, :mw], sc[:ksz, :mw], AF.Exp
                                )
                            else:
                                # single exp over both banks via a 2-region AP
                                nc.scalar.activation(
                                    es[:ksz, :580].rearrange(
                                        "p (a b) -> p a b", b=290
                                    ),
                                    sc[:ksz].rearrange("p (a b) -> p a b", a=2)[
                                        :, :, :290
                                    ],
                                    AF.Exp,
                                )
                            eng = nc.vector if hi == 0 else nc.gpsimd
                            eng.tensor_tensor(
                                es[:ksz, :mw],
                                es[:ksz, :mw],
                                mask_sb[kt][:ksz, :mw],
                                ALU.mult,
                            )
                            if dbg and b == 0 and h == 0 and kt == 0:
                                nc.sync.dma_start(dbg_t["dbg_es"], es[:])
                    oes = {}
                    for hi, h in enumerate((2 * hp, 2 * hp + 1)):
                        oes[hi] = [
                            poe.tile([65, csz], F32, tag=f"oe{ci}", name=f"oe{ci}")
                            for ci, (c0, csz) in enumerate(QCH)
                        ]
                        for kt, (k0, ksz) in enumerate(KT):
                            for ci, (c0, csz) in enumerate(QCH):
                                if kt == 1 and ci == 1:
                                    continue  # fully-masked block: adds zero
                                nc.tensor.matmul(
                                    oes[hi][ci][:, :],
                                    v_tok[kt][:ksz, h, :],
                                    es_t[hi][kt][:ksz, c0 : c0 + csz],
                                    start=(kt == 0),
                                    stop=(kt == len(KT) - 1),
                                )
                    for hi, h in enumerate((2 * hp, 2 * hp + 1)):
                        if dbg and b == 0 and h == 0:
                            oe_cp = rcpp.tile([65, 290], F32, tag="dbgoe")
                            nc.vector.tensor_copy(oe_cp[:], oes[hi][0][:, :290])
                            nc.sync.dma_start(dbg_t["dbg_oe"], oe_cp[:])
                        # stash unnormalized output + denominator row
                        sr = rcpp.tile([65, NP], F32, tag="sr")
                        for ci, (c0, csz) in enumerate(QCH):
                            nc.scalar.copy(
                                sr[64:65, c0 : c0 + csz], oes[hi][ci][64:65, :csz]
                            )
                        nc.sync.dma_start(srd[h : h + 1, :], sr[64:65, :])
                        dst_tile = oc_sb[h // 2]
                        if hi == 0:
                            for ci, (c0, csz) in enumerate(QCH):
                                nc.vector.tensor_copy(
                                    dst_tile[0:64, c0 : c0 + csz],
                                    oes[hi][ci][0:64, :csz],
                                )
                        else:
                            tmp = rcpp.tile([64, NP], F32R, tag="tmp")
                            for ci, (c0, csz) in enumerate(QCH):
                                nc.vector.tensor_copy(
                                    tmp[:, c0 : c0 + csz], oes[hi][ci][0:64, :csz]
                                )
                            nc.sync.dma_start(dst_tile[64:128, :], tmp[:, :])

                    if hp in (2, 5):
                        g0 = 6 * (hp // 3)  # heads g0..g0+5, pairs hp-2..hp
                        srs = rcpp.tile([6, NP], F32, tag="srs")
                        nc.sync.dma_start(srs[:], srd[g0 : g0 + 6, :])
                        rr = rcpp.tile([6, NP], F32, tag="rr")
                        nc.vector.reciprocal(rr[:], srs[:])
                        rrd = drp.tile([6, NP], F32, tag="rrd")
                        nc.sync.dma_start(rrd[:], rr[:])
                        for hp2 in range(hp - 2, hp + 1):
                            rb = rcpp.tile([P, NP], F32, tag="rb")
                            nc.sync.dma_start(
                                rb[0:64, :], _bcast_ap(rrd[2 * hp2 - g0], 64)
                            )
                            nc.sync.dma_start(
                                rb[64:128, :], _bcast_ap(rrd[2 * hp2 + 1 - g0], 64)
                            )
                            oc = oc_sb[hp2]
                            for po in (0, 64):
                                nc.vector.tensor_tensor(
                                    oc[po : po + 64, :],
                                    oc[po : po + 64, :],
                                    rb[po : po + 64, :],
                                    ALU.mult,
                                )
                            if dbg and b == 0 and hp2 == 0:
                                nc.sync.dma_start(dbg_t["dbg_rb"], rb[:])
                                nc.sync.dma_start(dbg_t["dbg_oc"], oc[:])

                # (normalization emitted inside the head-pair loop, above)

                # ---- prefetch next batch's x and v while norm completes ----
                prev_oc = oc_sb
                if b + 1 < BPC:
                    xT_b, v_tok = emit_xv(b + 1)
                emit_proj(b, prev_oc)

    nc.finalize()
    return nc


_CACHE = {}


def _make_in_maps(x, qkv_w, qkv_b, proj_w, proj_b):
    x = np.asarray(x, np.float32)
    qkv_w = np.asarray(qkv_w, np.float32)
    qkv_b = np.asarray(qkv_b, np.float32)
    proj_b = np.asarray(proj_b, np.float32)

    wqkT = np.ascontiguousarray(qkv_w[: 2 * C].T)
    wvT = np.ascontiguousarray(qkv_w[2 * C :].T)
    wpT = np.ascontiguousarray(np.asarray(proj_w, np.float32).T)
    bqk_h = qkv_b[: 2 * C].copy()
    bqk_h[:C] *= SCALE
    bv_h = np.ascontiguousarray(qkv_b[2 * C :])
    maskf = np.zeros((N, NP), np.float32)
    maskf[:, :N] = _build_mask_np()

    in_maps = []
    for c in range(NCORES):
        xp_c = np.zeros((BPC, NP, C), np.float32)
        xp_c[:, :N, :] = x[c * BPC : (c + 1) * BPC]
        xT_c = np.ascontiguousarray(xp_c.reshape(TP, C).T)
        in_maps.append(
            {
                "xT": xT_c,
                "wqkT": wqkT,
                "wvT": wvT,
                "wpT": wpT,
                "bqk": bqk_h,
                "bv": bv_h,
                "bp": proj_b,
                "maskf": maskf,
                "ones12": np.ones(H, np.float32),
            }
        )
    return in_maps


def kernel(x, qkv_w, qkv_b, proj_w, proj_b):
    if "nc" not in _CACHE:
        _CACHE["nc"] = _build_program()
    nc = _CACHE["nc"]

    in_maps = _make_in_maps(x, qkv_w, qkv_b, proj_w, proj_b)
    res = run_bass_kernel_spmd(nc, in_maps, list(range(NCORES)))
    out = np.concatenate(
        [res.results[c]["y"].reshape(BPC, N, C) for c in range(NCORES)], axis=0
    )
    return out.astype(np.float32)



# revision 7
# speedup vs baseline: 1.0468x; 1.0468x over previous
"""Trainium2 Bass kernel for CustomAttention (ViT-style windowed attention).

Math (per batch element):
  qkv = x @ qkv_w.T + qkv_b            -> q, k, v  [H=12 heads, D=64]
  s   = (q * D^-0.5) @ k.T             masked by a fixed 24x24-grid window
  attn = softmax(s)                    (CLS row/col always attended)
  out  = attn @ v                      -> concat heads -> @ proj_w.T + proj_b

Sharding: data-parallel over batch across 8 cores (4 images/core).

Key device-side choices:
  - The window mask in row-major token order is a band: patch key j is
    attended only by queries in [j-25, j+25] (plus CLS row/col). Scores and
    attn@v therefore run BANDED per 128-key tile: each key tile streams only
    a ~180-column query window instead of all 578, cutting PE/exp/mask work
    ~3x. Key tile 0 keeps the full query range (its row 0 is the CLS key,
    attended by every query). The CLS query (attends all keys) rides along as
    column 0 of each gathered query window; its attn@v contributions
    accumulate into output column 0 via N=1 matmuls.
  - attn@v accumulates banded segments into one [65, 580] PSUM tile spanning
    2 banks; matmuls split at the 512-column bank boundary and the first
    writer of each bank uses start=True (clears has_written for the bank).
  - All matmul operands are bf16 (fp32 PSUM accumulation): bf16 streams
    1 col/cycle at any N (fp32r needs N>=256), enables fast weight load, and
    halves DMA/SBUF. Verified max-rel error ~4e-3 vs the 2e-2 gate.
  - softmax runs unnormalized (no max subtraction; |s| small): exp then
    mask-multiply (split across DVE and GpSimd). v carries an interleaved
    ones column so attn@v also yields the softmax denominators (row 64).
  - normalization is deferred per head-pair: denominator rows collect in
    SBUF, one reciprocal [2, 578] computes inverses, broadcast back via a
    DRAM-source partition-stride-0 DMA, multiplied into the bf16 head
    outputs before proj.
  - a burst of junk warmup matmuls at kernel start keeps the PE HAM clock
    gate at full rate while weights stream in.
"""

import ml_dtypes
import numpy as np

import concourse.bass as bass
import concourse.mybir as mybir
from concourse import bacc
from concourse.bass_utils import run_bass_kernel_spmd
from concourse.tile import TileContext

B, N, C = 32, 577, 768
H, D = 12, 64
NCORES = 8
BPC = B // NCORES            # batches per core
NP = N + 1                   # padded token count (even)
TP = BPC * NP
T = BPC * N
SCALE = float(D) ** -0.5     # 0.125, exact in bf16
F32 = mybir.dt.float32
BF16 = mybir.dt.bfloat16
P = 128

CT = C // P                                      # 6 contraction tiles
KT = [(0, 128), (128, 128), (256, 128), (384, 128), (512, 65)]
QCH = [(0, 290), (290, 288)]                     # qk / kt0-score chunks
VCH = [(0, 512), (512, 256)]                     # v / proj output chunks
# banded q windows per key tile 1..4: (off, wlen, a, blen, k0, ksz)
# window col off+0 = CLS query, cols off+1..off+blen = q tokens [a, a+blen).
WIN = [
    (0, 179, 103, 178, 128, 128),
    (180, 179, 231, 178, 256, 128),
    (360, 179, 359, 178, 384, 128),
    (540, 91, 487, 90, 512, 65),
]
QGW = 720                    # gathered window tile width (windows at 180*wi)
AF = mybir.ActivationFunctionType
ALU = mybir.AluOpType


def _build_mask_np():
    img = 24
    p = np.arange(img * img)
    pi, pj = p // img, p % img
    ok = (np.abs(pi[:, None] - pi[None, :]) <= 1) & (
        np.abs(pj[:, None] - pj[None, :]) <= 1
    )
    m = np.zeros((N, N), dtype=np.float32)
    m[1:, 1:] = ok
    m[0, :] = True
    m[:, 0] = True
    return m


def _bcast_ap(ap1d, parts):
    """1-row AP -> [parts, n] with partition stride 0 (DRAM-source DMA)."""
    return bass.AP(
        tensor=ap1d.tensor, offset=ap1d.offset, ap=[[0, parts]] + list(ap1d.ap)[-1:]
    )


def _build_program():
    nc = bacc.Bacc("TRN2", target_bir_lowering=False, debug=False)
    xT = nc.dram_tensor("xT", [C, TP], BF16, kind="ExternalInput").ap()
    wqkT = nc.dram_tensor("wqkT", [C, 2 * C], BF16, kind="ExternalInput").ap()
    wvT = nc.dram_tensor("wvT", [C, C], BF16, kind="ExternalInput").ap()
    wpT = nc.dram_tensor("wpT", [C, C], BF16, kind="ExternalInput").ap()
    bqk = nc.dram_tensor("bqk", [2 * C], F32, kind="ExternalInput").ap()
    bv = nc.dram_tensor("bv", [C], F32, kind="ExternalInput").ap()
    bp = nc.dram_tensor("bp", [C], F32, kind="ExternalInput").ap()
    mask0d = nc.dram_tensor("mask0d", [P, NP], BF16, kind="ExternalInput").ap()
    maskwd = nc.dram_tensor("maskwd", [P, QGW], BF16, kind="ExternalInput").ap()
    ones12 = nc.dram_tensor("ones12", [H], BF16, kind="ExternalInput").ap()
    y = nc.dram_tensor("y", [T, C], F32, kind="ExternalOutput").ap()

    with TileContext(nc) as tc:
        with (
            tc.tile_pool(name="singles", bufs=1) as singles,
            tc.tile_pool(name="xp", bufs=2) as xp,
            tc.tile_pool(name="qkp", bufs=2) as qkp,
            tc.tile_pool(name="qgp", bufs=2) as qgp,
            tc.tile_pool(name="vtp", bufs=2) as vtp,
            tc.tile_pool(name="ocp", bufs=2) as ocp,
            tc.tile_pool(name="esp", bufs=2) as esp,
            tc.tile_pool(name="esw", bufs=8) as eswp,
            tc.tile_pool(name="rcp", bufs=2) as rcpp,
            tc.tile_pool(name="ysp", bufs=2) as ysp,
            tc.tile_pool(name="pmm", bufs=2, space="PSUM") as pmm,
            tc.tile_pool(name="psc", bufs=4, space="PSUM") as psc,
            tc.tile_pool(name="poe", bufs=1, space="PSUM") as poe,
            tc.tile_pool(name="drp", bufs=4, space="DRAM") as drp,
        ):
            # ---- PE warmup: junk matmuls overlap the weight DMAs and get
            # the HAM clock gate to 8/8 before real work starts ----
            wup = singles.tile([P, 512], BF16, tag="wup")
            nc.vector.memset(wup[:], 1.0)
            for i in range(24):
                ps = pmm.tile([P, 512], F32, tag="pb", name="ps")
                nc.tensor.matmul(ps[:, :512], wup[:, 0:P], wup[:, 0:512],
                                 start=True, stop=True, skip_group_check=True)

            # ---- persistent loads ----
            wqk_sb = []
            wv_sb = []
            wp_sb = []
            for ct in range(CT):
                t = singles.tile([P, 2 * C], BF16, tag=f"wqk{ct}")
                nc.sync.dma_start(t[:], wqkT[ct * P : (ct + 1) * P, :])
                wqk_sb.append(t)
                t = singles.tile([P, C], BF16, tag=f"wv{ct}")
                nc.sync.dma_start(t[:], wvT[ct * P : (ct + 1) * P, :])
                wv_sb.append(t)
                t = singles.tile([P, C], BF16, tag=f"wp{ct}")
                nc.sync.dma_start(t[:], wpT[ct * P : (ct + 1) * P, :])
                wp_sb.append(t)
            bqk_sb = singles.tile([P, 2 * C // P], F32, tag="bqk")
            nc.sync.dma_start(bqk_sb[:], bqk.rearrange("(o p) -> p o", p=P))
            bv_sb = singles.tile([P, C], F32, tag="bv")
            nc.sync.dma_start(bv_sb[:], _bcast_ap(bv, P))
            bp_sb = singles.tile([P, C], F32, tag="bp")
            nc.sync.dma_start(bp_sb[:], _bcast_ap(bp, P))
            ones_sb = singles.tile([P, H], BF16, tag="ones_sb")
            nc.sync.dma_start(ones_sb[:], _bcast_ap(ones12, P))
            mask0_sb = singles.tile([P, NP], BF16, tag="mask0")
            nc.sync.dma_start(mask0_sb[:], mask0d[:, :])
            maskw_sb = singles.tile([P, QGW], BF16, tag="maskw")
            nc.sync.dma_start(maskw_sb[:], maskwd[:, :])

            def emit_x_dma(b):
                xT_b = []
                for ct in range(CT):
                    t = xp.tile([P, NP], BF16, tag=f"x{ct}", name=f"x{ct}")
                    nc.sync.dma_start(
                        t[:], xT[ct * P : (ct + 1) * P, b * NP : (b + 1) * NP]
                    )
                    xT_b.append(t)
                return xT_b

            def emit_v(xT_b):
                v_tok = []
                for mt, (m0, msz) in enumerate(KT):
                    vt = vtp.tile([P, H, D + 1], BF16, tag=f"vt{mt}", name=f"vt{mt}")
                    nc.vector.tensor_copy(vt[:, :, D : D + 1], ones_sb[:, :, None])
                    pss = [
                        pmm.tile([P, 512], F32, tag="pb", name="ps")
                        for _ in range(2)
                    ]
                    for ct in range(CT):
                        for ci, (c0, csz) in enumerate(VCH):
                            nc.tensor.matmul(
                                pss[ci][:msz, :csz],
                                xT_b[ct][:, m0 : m0 + msz],
                                wv_sb[ct][:, c0 : c0 + csz],
                                start=(ct == 0),
                                stop=(ct == CT - 1),
                            )
                    for ci, (c0, csz) in enumerate(VCH):
                        nh = csz // D
                        h0 = c0 // D
                        nc.vector.tensor_tensor(
                            vt[:msz, h0 : h0 + nh, 0:D],
                            pss[ci][:msz, :csz].rearrange("p (h d) -> p h d", d=D),
                            bv_sb[:msz, c0 : c0 + csz].rearrange(
                                "p (h d) -> p h d", d=D
                            ),
                            ALU.add,
                        )
                    v_tok.append(vt)
                return v_tok

            def emit_qk(hp, xT_b):
                qt = qkp.tile([P, NP], BF16, tag="qk_q")
                ktb = qkp.tile([P, NP], BF16, tag="qk_k")
                qg = qgp.tile([P, QGW], BF16, tag="qg")
                for dst, ft in ((qt, hp), (ktb, CT + hp)):
                    pss = [
                        pmm.tile([P, 512], F32, tag="pb", name="ps")
                        for _ in range(2)
                    ]
                    for ct in range(CT):
                        for ci, (c0, csz) in enumerate(QCH):
                            nc.tensor.matmul(
                                pss[ci][:, :csz],
                                wqk_sb[ct][:, ft * P : (ft + 1) * P],
                                xT_b[ct][:, c0 : c0 + csz],
                                start=(ct == 0),
                                stop=(ct == CT - 1),
                            )
                    for ci, (c0, csz) in enumerate(QCH):
                        nc.vector.scalar_tensor_tensor(
                            dst[:, c0 : c0 + csz],
                            pss[ci][:, :csz],
                            1.0,
                            bqk_sb[:, ft : ft + 1].to_broadcast([P, csz]),
                            ALU.mult,
                            ALU.add,
                        )
                # gather banded q windows (+ CLS col at each window start)
                for off, wlen, a, blen, k0, ksz in WIN:
                    nc.gpsimd.tensor_copy(
                        qg[:, off + 1 : off + 1 + blen], qt[:, a : a + blen]
                    )
                nc.gpsimd.tensor_copy(
                    qg[:, 0:QGW].rearrange("p (w c) -> p w c", c=180)[:, :, 0:1],
                    qt[:, 0:1][:, None, :].to_broadcast([P, 4, 1]),
                )
                return qt, ktb, qg

            def emit_scores(qt, ktb, qg):
                """scores -> exp -> mask for both heads of the pair.
                Returns es0[hi], esw[hi][wi] bf16 tiles."""
                es0 = {}
                esw = {0: [], 1: []}
                for hi in (0, 1):
                    es0[hi] = esp.tile([P, NP + 2], BF16, tag="es0", name="es0")
                # kt0: dense query range, 2 chunks per head
                for hi in (0, 1):
                    po = D * hi
                    for ci, (c0, csz) in enumerate(QCH):
                        sc = psc.tile([P, 512], F32, tag="sc", name="sc")
                        nc.tensor.matmul(
                            sc[:, :csz],
                            ktb[po : po + D, 0:128],
                            qt[po : po + D, c0 : c0 + csz],
                            start=True,
                            stop=True,
                        )
                        nc.scalar.activation(
                            es0[hi][:, c0 : c0 + csz], sc[:, :csz], AF.Exp
                        )
                    eng = nc.vector if hi == 0 else nc.gpsimd
                    eng.tensor_tensor(
                        es0[hi][:, 0:N], es0[hi][:, 0:N], mask0_sb[:, 0:N], ALU.mult
                    )
                # kt1..4: banded windows
                for wi, (off, wlen, a, blen, k0, ksz) in enumerate(WIN):
                    for hi in (0, 1):
                        po = D * hi
                        sc = psc.tile([P, 512], F32, tag="sc", name="sc")
                        nc.tensor.matmul(
                            sc[:ksz, :wlen],
                            ktb[po : po + D, k0 : k0 + ksz],
                            qg[po : po + D, off : off + wlen],
                            start=True,
                            stop=True,
                        )
                        es = eswp.tile([P, 180], BF16, tag="esw", name="esw")
                        esw[hi].append(es)
                        nc.scalar.activation(es[:ksz, :wlen], sc[:ksz, :wlen], AF.Exp)
                        eng = nc.vector if hi == 0 else nc.gpsimd
                        eng.tensor_tensor(
                            es[:ksz, :wlen],
                            es[:ksz, :wlen],
                            maskw_sb[:ksz, off : off + wlen],
                            ALU.mult,
                        )
                return es0, esw

            def emit_av(hp, hi, es0, esw, v_tok, oc_sb, srs):
                """banded attn@v for head h, stash output + denominator."""
                h = 2 * hp + hi
                oe = poe.tile([D + 1, NP + 2], F32, tag="oe", name="oe")
                mm = nc.tensor.matmul
                # kt0 dense: first writer of both PSUM banks (start=True)
                mm(oe[:, 0:512], v_tok[0][:, h, :], es0[hi][:, 0:512],
                   start=True, stop=False, skip_group_check=True)
                mm(oe[:, 512:N], v_tok[0][:, h, :], es0[hi][:, 512:N],
                   start=True, stop=False, skip_group_check=True)
                # banded tiles: accumulate segments (split at bank boundary)
                for wi, (off, wlen, a, blen, k0, ksz) in enumerate(WIN):
                    es = esw[hi][wi]
                    vkt = v_tok[wi + 1][:ksz, h, :]
                    if a + blen <= 512:
                        segs = [(1, 1 + blen, a)]
                    else:
                        sp = 512 - a + 1
                        segs = [(1, sp, a), (sp, 1 + blen, 512)]
                    for s0, s1, o0 in segs:
                        mm(oe[:, o0 : o0 + (s1 - s0)], vkt, es[:ksz, s0:s1],
                           start=False, stop=False, skip_group_check=True)
                    # CLS query column accumulates into output column 0
                    mm(oe[:, 0:1], vkt, es[:ksz, 0:1],
                       start=False, stop=(wi == len(WIN) - 1),
                       skip_group_check=True)
                # denominator row -> srs row hi (via sbuf-sbuf DMA)
                sr = rcpp.tile([D + 1, NP], F32, tag="sr")
                nc.scalar.copy(sr[D : D + 1, 0:N], oe[D : D + 1, 0:N])
                nc.sync.dma_start(srs[hi : hi + 1, 0:N], sr[D : D + 1, 0:N])
                # unnormalized head output -> oc (bf16)
                dst_tile = oc_sb[hp]
                if hi == 0:
                    nc.vector.tensor_copy(dst_tile[0:D, 0:N], oe[0:D, 0:N])
                else:
                    tmp = rcpp.tile([D, NP], BF16, tag="tmp")
                    nc.vector.tensor_copy(tmp[:, 0:N], oe[0:D, 0:N])
                    nc.sync.dma_start(dst_tile[D : 2 * D, 0:N], tmp[:, 0:N])

            def emit_norm(hp, srs, oc_sb):
                """reciprocal of the pair's denominators, broadcast, apply."""
                rr = rcpp.tile([2, NP], BF16, tag="rr")
                with nc.allow_low_precision(reason="bf16 1/denom, gate is 2e-2"):
                    nc.vector.reciprocal(rr[:, 0:N], srs[:, 0:N])
                rrd = drp.tile([2, NP], BF16, tag="rrd")
                nc.sync.dma_start(rrd[:, :], rr[:, :])
                rb = rcpp.tile([P, NP], BF16, tag="rb")
                nc.sync.dma_start(rb[0:D, 0:N], _bcast_ap(rrd[0][0:N], D))
                nc.sync.dma_start(rb[D : 2 * D, 0:N], _bcast_ap(rrd[1][0:N], D))
                oc = oc_sb[hp]
                for po in (0, D):
                    nc.vector.tensor_tensor(
                        oc[po : po + D, 0:N],
                        oc[po : po + D, 0:N],
                        rb[po : po + D, 0:N],
                        ALU.mult,
                    )

            def emit_proj(b, oc_sb):
                for mt, (m0, msz) in enumerate(KT):
                    ysb = ysp.tile([P, C], F32, tag="ysb", name="ysb")
                    pss = [
                        pmm.tile([P, 512], F32, tag="pb", name="ps")
                        for _ in range(2)
                    ]
                    for ct in range(CT):
                        for ci, (c0, csz) in enumerate(VCH):
                            nc.tensor.matmul(
                                pss[ci][:msz, :csz],
                                oc_sb[ct][:, m0 : m0 + msz],
                                wp_sb[ct][:, c0 : c0 + csz],
                                start=(ct == 0),
                                stop=(ct == CT - 1),
                            )
                    for ci, (c0, csz) in enumerate(VCH):
                        nc.vector.tensor_tensor(
                            ysb[:msz, c0 : c0 + csz],
                            pss[ci][:msz, :csz],
                            bp_sb[:msz, c0 : c0 + csz],
                            ALU.add,
                        )
                    nc.sync.dma_start(
                        y[b * N + m0 : b * N + m0 + msz, :], ysb[:msz, :]
                    )

            # ---- main schedule (software-pipelined across pairs/batches) ----
            xT_b = emit_x_dma(0)
            v_tok = emit_v(xT_b)
            for b in range(BPC):
                oc_sb = [
                    ocp.tile([P, NP], BF16, tag=f"oc{ct}", name=f"oc{ct}")
                    for ct in range(CT)
                ]
                srs_l = []
                qk_t = emit_qk(0, xT_b)
                nxt_x = None
                for hp in range(H // 2):
                    qt, ktb, qg = qk_t
                    es0, esw = emit_scores(qt, ktb, qg)
                    if hp == 2 and b + 1 < BPC:
                        nxt_x = emit_x_dma(b + 1)
                    if hp + 1 < H // 2:
                        qk_t = emit_qk(hp + 1, xT_b)
                    srs = rcpp.tile([2, NP], F32, tag="srs")
                    srs_l.append(srs)
                    if hp >= 1:
                        emit_norm(hp - 1, srs_l[hp - 1], oc_sb)
                    for hi in (0, 1):
                        emit_av(hp, hi, es0, esw, v_tok, oc_sb, srs)
                prev_oc = oc_sb
                prev_srs = srs_l[H // 2 - 1]
                if b + 1 < BPC:
                    xT_b = nxt_x
                    v_tok = emit_v(xT_b)
                emit_norm(H // 2 - 1, prev_srs, prev_oc)
                emit_proj(b, prev_oc)

    nc.finalize()
    return nc


_CACHE = {}


def _make_in_maps(x, qkv_w, qkv_b, proj_w, proj_b):
    bf = ml_dtypes.bfloat16
    x = np.asarray(x, np.float32)
    qkv_w = np.asarray(qkv_w, np.float32)
    qkv_b = np.asarray(qkv_b, np.float32)
    proj_w = np.asarray(proj_w, np.float32)
    proj_b = np.asarray(proj_b, np.float32)

    wqkT = np.ascontiguousarray(qkv_w[: 2 * C].T).copy()
    wqkT[:, :C] *= SCALE
    wqkT = wqkT.astype(bf)
    wvT = np.ascontiguousarray(qkv_w[2 * C :].T).astype(bf)
    wpT = np.ascontiguousarray(proj_w.T).astype(bf)
    bqk_h = qkv_b[: 2 * C].copy()
    bqk_h[:C] *= SCALE
    bv_h = np.ascontiguousarray(qkv_b[2 * C :])

    m = _build_mask_np()
    mask0 = np.zeros((P, NP), np.float32)
    mask0[:, :N] = m[:P, :]
    mask0 = mask0.astype(bf)
    maskw = np.zeros((P, QGW), np.float32)
    for off, wlen, a, blen, k0, ksz in WIN:
        maskw[:ksz, off] = 1.0
        maskw[:ksz, off + 1 : off + 1 + blen] = m[k0 : k0 + ksz, a : a + blen]
    maskw = maskw.astype(bf)

    in_maps = []
    for c in range(NCORES):
        xp_c = np.zeros((BPC, NP, C), np.float32)
        xp_c[:, :N, :] = x[c * BPC : (c + 1) * BPC]
        xT_c = np.ascontiguousarray(xp_c.reshape(TP, C).T).astype(bf)
        in_maps.append(
            {
                "xT": xT_c,
                "wqkT": wqkT,
                "wvT": wvT,
                "wpT": wpT,
                "bqk": bqk_h,
                "bv": bv_h,
                "bp": proj_b,
                "mask0d": mask0,
                "maskwd": maskw,
                "ones12": np.ones(H, bf),
            }
        )
    return in_maps


def kernel(x, qkv_w, qkv_b, proj_w, proj_b):
    if "nc" not in _CACHE:
        _CACHE["nc"] = _build_program()
    nc = _CACHE["nc"]

    in_maps = _make_in_maps(x, qkv_w, qkv_b, proj_w, proj_b)
    res = run_bass_kernel_spmd(nc, in_maps, list(range(NCORES)))
    out = np.concatenate(
        [res.results[c]["y"].reshape(BPC, N, C) for c in range(NCORES)], axis=0
    )
    return out.astype(np.float32)


# revision 15
# speedup vs baseline: 1.4106x; 1.3475x over previous
"""Trainium2 Bass kernel for CustomAttention (ViT-style windowed attention).

Math (per batch element):
  qkv = x @ qkv_w.T + qkv_b            -> q, k, v  [H=12 heads, D=64]
  s   = (q * D^-0.5) @ k.T             masked by a fixed 24x24-grid window
  attn = softmax(s)                    (CLS row/col always attended)
  out  = attn @ v                      -> concat heads -> @ proj_w.T + proj_b

Sharding: data-parallel over batch across 8 cores (4 images/core).

Key device-side choices:
  - The window mask in row-major token order is a band: patch key j is
    attended only by queries in [j-25, j+25] (plus CLS row/col). Scores and
    attn@v therefore run BANDED per 128-key tile: each key tile streams only
    its ~180-column query window instead of all 578. Key tile 0 keeps the
    full query range (its row 0 is the CLS key, attended by every query).
    The CLS query (attends all keys) lands in column 0 of each window via
    N=1 matmuls; its attn@v contributions accumulate into output column 0.
  - Window score tiles are packed pairwise into one PSUM bank (kt1+kt2,
    kt3+kt4) so exp and mask run once per packed tile.
  - attn@v accumulates banded segments into one [65, 580] PSUM tile spanning
    2 banks; matmuls split at the 512-column bank boundary and the first
    writer of each bank uses start=True (clears has_written for the bank).
  - All matmul operands are bf16 (fp32 PSUM accumulation): bf16 streams
    1 col/cycle at any N (fp32r needs N>=256), enables fast weight load, and
    halves DMA/SBUF. Measured max-rel error ~4e-3 vs the 2e-2 gate.
  - softmax runs unnormalized (no max subtraction; |s| small): exp then
    mask-multiply. v carries an interleaved ones column so attn@v also
    yields the softmax denominators (row 64). Head outputs + denominators
    stage through one bf16 copy; normalization is deferred one pair:
    Scalar-engine reciprocal, DRAM-source partition-broadcast DMA, bf16
    multiply before proj.
  - a burst of junk warmup matmuls at kernel start keeps the PE HAM clock
    gate at full rate while x/weights stream in (x DMAs issued first).
"""

import ml_dtypes
import numpy as np

import concourse.bass as bass
import concourse.mybir as mybir
from concourse import bacc
from concourse.bass_utils import run_bass_kernel_spmd
from concourse.tile import TileContext

B, N, C = 32, 577, 768
H, D = 12, 64
NCORES = 8
BPC = B // NCORES            # batches per core
NP = N + 1                   # padded token count (even)
TP = BPC * NP
T = BPC * N
SCALE = float(D) ** -0.5     # 0.125, exact in bf16
F32 = mybir.dt.float32
BF16 = mybir.dt.bfloat16
P = 128

CT = C // P                                      # 6 contraction tiles
KT = [(0, 128), (128, 128), (256, 128), (384, 128), (512, 65)]
QCH = [(0, 290), (290, 288)]                     # qk / kt0-score chunks
VCH = [(0, 512), (512, 256)]                     # v / proj output chunks
# banded windows for key tiles 1..4: (grp, off, a, blen, k0, ksz)
# grp selects the packed psum/es tile (0: kt1+kt2, 1: kt3+kt4); within it,
# cols off+0/off+1 = scores vs q tokens 0 (CLS) and 1 (masked to zero),
# cols off+2..off+1+blen = q tokens [a, a+blen). All col offsets/sizes even.
WIN = [
    (0, 0, 102, 180, 128, 128),
    (0, 182, 230, 180, 256, 128),
    (1, 0, 358, 180, 384, 128),
    (1, 182, 486, 92, 512, 65),
]
GW = [364, 276]              # packed window tile widths
MW = sum(GW)                 # banded mask tile width
AF = mybir.ActivationFunctionType
ALU = mybir.AluOpType


def _build_mask_np():
    img = 24
    p = np.arange(img * img)
    pi, pj = p // img, p % img
    ok = (np.abs(pi[:, None] - pi[None, :]) <= 1) & (
        np.abs(pj[:, None] - pj[None, :]) <= 1
    )
    m = np.zeros((N, N), dtype=np.float32)
    m[1:, 1:] = ok
    m[0, :] = True
    m[:, 0] = True
    return m


def _bcast_ap(ap1d, parts):
    """1-row AP -> [parts, n] with partition stride 0 (DRAM-source DMA)."""
    return bass.AP(
        tensor=ap1d.tensor, offset=ap1d.offset, ap=[[0, parts]] + list(ap1d.ap)[-1:]
    )


def _build_program():
    nc = bacc.Bacc("TRN2", target_bir_lowering=False, debug=False)
    xT = nc.dram_tensor("xT", [C, TP], BF16, kind="ExternalInput").ap()
    wqkT = nc.dram_tensor("wqkT", [C, 2 * C], BF16, kind="ExternalInput").ap()
    wvT = nc.dram_tensor("wvT", [C, C], BF16, kind="ExternalInput").ap()
    wpT = nc.dram_tensor("wpT", [C, C], BF16, kind="ExternalInput").ap()
    bqk = nc.dram_tensor("bqk", [2 * C], F32, kind="ExternalInput").ap()
    bv = nc.dram_tensor("bv", [C], F32, kind="ExternalInput").ap()
    bp = nc.dram_tensor("bp", [C], F32, kind="ExternalInput").ap()
    mask0d = nc.dram_tensor("mask0d", [P, NP], BF16, kind="ExternalInput").ap()
    maskwd = nc.dram_tensor("maskwd", [P, MW], BF16, kind="ExternalInput").ap()
    ones12 = nc.dram_tensor("ones12", [H], BF16, kind="ExternalInput").ap()
    y = nc.dram_tensor("y", [T, C], F32, kind="ExternalOutput").ap()

    with TileContext(nc) as tc:
        with (
            tc.tile_pool(name="singles", bufs=1) as singles,
            tc.tile_pool(name="xp", bufs=2) as xp,
            tc.tile_pool(name="qkp", bufs=2) as qkp,
            tc.tile_pool(name="vtp", bufs=2) as vtp,
            tc.tile_pool(name="ocp", bufs=2) as ocp,
            tc.tile_pool(name="esp", bufs=2) as esp,
            tc.tile_pool(name="esw", bufs=4) as eswp,
            tc.tile_pool(name="rcp", bufs=3) as rcpp,
            tc.tile_pool(name="ysp", bufs=2) as ysp,
            tc.tile_pool(name="pmm", bufs=2, space="PSUM") as pmm,
            tc.tile_pool(name="psc", bufs=4, space="PSUM") as psc,
            tc.tile_pool(name="poe", bufs=1, space="PSUM") as poe,
            tc.tile_pool(name="drp", bufs=4, space="DRAM") as drp,
        ):
            # ---- prefetch batch 0's x before the weights ----
            def emit_x_dma(b):
                xT_b = []
                for ct in range(CT):
                    t = xp.tile([P, NP], BF16, tag=f"x{ct}", name=f"x{ct}")
                    nc.sync.dma_start(
                        t[:], xT[ct * P : (ct + 1) * P, b * NP : (b + 1) * NP]
                    )
                    xT_b.append(t)
                return xT_b

            xT_b = emit_x_dma(0)

            # ---- PE warmup: junk matmuls overlap the input DMAs and get
            # the HAM clock gate to 8/8 before real work starts ----
            wup = singles.tile([P, 512], BF16, tag="wup")
            nc.vector.memset(wup[:], 1.0)
            for i in range(24):
                ps = pmm.tile([P, 512], F32, tag="pb", name="ps")
                nc.tensor.matmul(ps[:, :512], wup[:, 0:P], wup[:, 0:512],
                                 start=True, stop=True, skip_group_check=True)

            # ---- persistent loads (v/qk weights first) ----
            wv_sb = []
            wqk_sb = []
            wp_sb = []
            for ct in range(CT):
                t = singles.tile([P, C], BF16, tag=f"wv{ct}")
                nc.sync.dma_start(t[:], wvT[ct * P : (ct + 1) * P, :])
                wv_sb.append(t)
            for ct in range(CT):
                t = singles.tile([P, 2 * C], BF16, tag=f"wqk{ct}")
                nc.sync.dma_start(t[:], wqkT[ct * P : (ct + 1) * P, :])
                wqk_sb.append(t)
            bqk_sb = singles.tile([P, 2 * C // P], F32, tag="bqk")
            nc.sync.dma_start(bqk_sb[:], bqk.rearrange("(o p) -> p o", p=P))
            bv_sb = singles.tile([P, C], F32, tag="bv")
            nc.sync.dma_start(bv_sb[:], _bcast_ap(bv, P))
            ones_sb = singles.tile([P, H], BF16, tag="ones_sb")
            nc.sync.dma_start(ones_sb[:], _bcast_ap(ones12, P))
            mask0_sb = singles.tile([P, NP], BF16, tag="mask0")
            nc.sync.dma_start(mask0_sb[:], mask0d[:, :])
            maskw_sb = singles.tile([P, MW], BF16, tag="maskw")
            nc.sync.dma_start(maskw_sb[:], maskwd[:, :])
            for ct in range(CT):
                t = singles.tile([P, C], BF16, tag=f"wp{ct}")
                nc.sync.dma_start(t[:], wpT[ct * P : (ct + 1) * P, :])
                wp_sb.append(t)
            bp_sb = singles.tile([P, C], F32, tag="bp")
            nc.sync.dma_start(bp_sb[:], _bcast_ap(bp, P))

            def emit_v(xT_b):
                v_tok = []
                for mt, (m0, msz) in enumerate(KT):
                    vt = vtp.tile([P, H, D + 1], BF16, tag=f"vt{mt}", name=f"vt{mt}")
                    nc.vector.tensor_copy(vt[:, :, D : D + 1], ones_sb[:, :, None])
                    pss = [
                        pmm.tile([P, 512], F32, tag="pb", name="ps")
                        for _ in range(2)
                    ]
                    for ct in range(CT):
                        for ci, (c0, csz) in enumerate(VCH):
                            nc.tensor.matmul(
                                pss[ci][:msz, :csz],
                                xT_b[ct][:, m0 : m0 + msz],
                                wv_sb[ct][:, c0 : c0 + csz],
                                start=(ct == 0),
                                stop=(ct == CT - 1),
                            )
                    for ci, (c0, csz) in enumerate(VCH):
                        nh = csz // D
                        h0 = c0 // D
                        nc.vector.tensor_tensor(
                            vt[:msz, h0 : h0 + nh, 0:D],
                            pss[ci][:msz, :csz].rearrange("p (h d) -> p h d", d=D),
                            bv_sb[:msz, c0 : c0 + csz].rearrange(
                                "p (h d) -> p h d", d=D
                            ),
                            ALU.add,
                        )
                    v_tok.append(vt)
                return v_tok

            def emit_qk(hp, xT_b):
                qt = qkp.tile([P, NP], BF16, tag="qk_q")
                ktb = qkp.tile([P, NP], BF16, tag="qk_k")
                for dst, ft in ((qt, hp), (ktb, CT + hp)):
                    pss = [
                        pmm.tile([P, 512], F32, tag="pb", name="ps")
                        for _ in range(2)
                    ]
                    for ct in range(CT):
                        for ci, (c0, csz) in enumerate(QCH):
                            nc.tensor.matmul(
                                pss[ci][:, :csz],
                                wqk_sb[ct][:, ft * P : (ft + 1) * P],
                                xT_b[ct][:, c0 : c0 + csz],
                                start=(ct == 0),
                                stop=(ct == CT - 1),
                            )
                    for ci, (c0, csz) in enumerate(QCH):
                        nc.vector.scalar_tensor_tensor(
                            dst[:, c0 : c0 + csz],
                            pss[ci][:, :csz],
                            1.0,
                            bqk_sb[:, ft : ft + 1].to_broadcast([P, csz]),
                            ALU.mult,
                            ALU.add,
                        )
                return qt, ktb

            def emit_scores(qt, ktb):
                """scores -> exp -> mask for both heads of the pair.
                Returns es0[hi] (kt0, dense) and esg[hi][grp] (packed
                windows) bf16 tiles."""
                es0 = {}
                esg = {0: [None, None], 1: [None, None]}
                mm = nc.tensor.matmul
                for hi in (0, 1):
                    es0[hi] = esp.tile([P, NP + 2], BF16, tag="es0", name="es0")
                # kt0: dense query range, 2 chunks per head
                for hi in (0, 1):
                    po = D * hi
                    for ci, (c0, csz) in enumerate(QCH):
                        sc = psc.tile([P, 512], F32, tag="sc", name="sc")
                        mm(
                            sc[:, :csz],
                            ktb[po : po + D, 0:128],
                            qt[po : po + D, c0 : c0 + csz],
                            start=True,
                            stop=True,
                        )
                        nc.scalar.activation(
                            es0[hi][:, c0 : c0 + csz], sc[:, :csz], AF.Exp
                        )
                    eng = nc.vector if hi == 0 else nc.gpsimd
                    eng.tensor_tensor(
                        es0[hi][:, 0:N], es0[hi][:, 0:N], mask0_sb[:, 0:N], ALU.mult
                    )
                # kt1..4: banded windows, packed 2 per psum tile
                for grp in (0, 1):
                    wins = WIN[2 * grp : 2 * grp + 2]
                    for hi in (0, 1):
                        po = D * hi
                        sc = psc.tile([P, 512], F32, tag="sc", name="sc")
                        for g, off, a, blen, k0, ksz in wins:
                            mm(
                                sc[:ksz, off + 2 : off + 2 + blen],
                                ktb[po : po + D, k0 : k0 + ksz],
                                qt[po : po + D, a : a + blen],
                                start=True, stop=True, skip_group_check=True,
                            )
                            mm(
                                sc[:ksz, off : off + 2],
                                ktb[po : po + D, k0 : k0 + ksz],
                                qt[po : po + D, 0:2],
                                start=True, stop=True, skip_group_check=True,
                            )
                        gw = GW[grp]
                        es = eswp.tile([P, GW[0]], BF16, tag=f"esg{grp}",
                                       name=f"esg{grp}")
                        esg[hi][grp] = es
                        nc.scalar.activation(es[:, :gw], sc[:, :gw], AF.Exp)
                        eng = nc.vector if hi == 0 else nc.gpsimd
                        m0 = grp * GW[0]
                        eng.tensor_tensor(
                            es[:, :gw], es[:, :gw],
                            maskw_sb[:, m0 : m0 + gw], ALU.mult,
                        )
                return es0, esg

            def emit_av(hp, hi, es0, esg, v_tok, oc_sb, srs):
                """banded attn@v for head h, stage output + denominator."""
                h = 2 * hp + hi
                oe = poe.tile([D + 1, NP + 2], F32, tag="oe", name="oe")
                mm = nc.tensor.matmul
                # kt0 dense: first writer of both PSUM banks (start=True)
                mm(oe[:, 0:512], v_tok[0][:, h, :], es0[hi][:, 0:512],
                   start=True, stop=False, skip_group_check=True)
                mm(oe[:, 512:N], v_tok[0][:, h, :], es0[hi][:, 512:N],
                   start=True, stop=False, skip_group_check=True)
                # banded tiles: accumulate segments (split at bank boundary)
                for wi, (grp, off, a, blen, k0, ksz) in enumerate(WIN):
                    es = esg[hi][grp]
                    vkt = v_tok[wi + 1][:ksz, h, :]
                    s0 = off + 2
                    if a + blen <= 512:
                        segs = [(s0, s0 + blen, a)]
                    else:
                        sp = s0 + (512 - a)
                        segs = [(s0, sp, a), (sp, s0 + blen, 512)]
                    for g0, g1, o0 in segs:
                        mm(oe[:, o0 : o0 + (g1 - g0)], vkt, es[:ksz, g0:g1],
                           start=False, stop=False, skip_group_check=True)
                    # CLS query column accumulates into output column 0
                    # (column 1 adds masked zeros)
                    mm(oe[:, 0:2], vkt, es[:ksz, off : off + 2],
                       start=False, stop=(wi == len(WIN) - 1),
                       skip_group_check=True)
                # stage head output rows; extract fp32 denominator row
                stage = rcpp.tile([D, NP], BF16, tag="stage")
                nc.vector.tensor_copy(stage[:, 0:N], oe[0:D, 0:N])
                nc.sync.dma_start(
                    oc_sb[hp][D * hi : D * hi + D, 0:N], stage[:, 0:N]
                )
                srf = rcpp.tile([D + 1, NP], F32, tag="srf")
                nc.scalar.copy(srf[D : D + 1, 0:N], oe[D : D + 1, 0:N])
                nc.sync.dma_start(srs[hi : hi + 1, 0:N], srf[D : D + 1, 0:N])

            def emit_norm(hp, srs, oc_sb):
                """reciprocal of the pair's denominators, broadcast, apply."""
                rr = rcpp.tile([2, NP], F32, tag="rr")
                nc.vector.reciprocal_approx_fast(rr[:, 0:N], srs[:, 0:N])
                rrd = drp.tile([2, NP], F32, tag="rrd")
                nc.sync.dma_start(rrd[:, :], rr[:, :])
                rb = rcpp.tile([P, NP], F32, tag="rb")
                nc.sync.dma_start(rb[0:D, 0:N], _bcast_ap(rrd[0][0:N], D))
                nc.sync.dma_start(rb[D : 2 * D, 0:N], _bcast_ap(rrd[1][0:N], D))
                oc = oc_sb[hp]
                for po in (0, D):
                    nc.vector.tensor_tensor(
                        oc[po : po + D, 0:N],
                        oc[po : po + D, 0:N],
                        rb[po : po + D, 0:N],
                        ALU.mult,
                    )

            def emit_proj(b, oc_sb):
                for mt, (m0, msz) in enumerate(KT):
                    ysb = ysp.tile([P, C], F32, tag="ysb", name="ysb")
                    pss = [
                        pmm.tile([P, 512], F32, tag="pb", name="ps")
                        for _ in range(2)
                    ]
                    for ct in range(CT):
                        for ci, (c0, csz) in enumerate(VCH):
                            nc.tensor.matmul(
                                pss[ci][:msz, :csz],
                                oc_sb[ct][:, m0 : m0 + msz],
                                wp_sb[ct][:, c0 : c0 + csz],
                                start=(ct == 0),
                                stop=(ct == CT - 1),
                            )
                    for ci, (c0, csz) in enumerate(VCH):
                        nc.vector.tensor_tensor(
                            ysb[:msz, c0 : c0 + csz],
                            pss[ci][:msz, :csz],
                            bp_sb[:msz, c0 : c0 + csz],
                            ALU.add,
                        )
                    nc.sync.dma_start(
                        y[b * N + m0 : b * N + m0 + msz, :], ysb[:msz, :]
                    )

            # ---- main schedule (software-pipelined across pairs/batches) ----
            v_tok = emit_v(xT_b)
            for b in range(BPC):
                oc_sb = [
                    ocp.tile([P, NP], BF16, tag=f"oc{ct}", name=f"oc{ct}")
                    for ct in range(CT)
                ]
                srs_l = []
                qk_t = emit_qk(0, xT_b)
                nxt_x = None
                for hp in range(H // 2):
                    qt, ktb = qk_t
                    es0, esg = emit_scores(qt, ktb)
                    if hp == 2 and b + 1 < BPC:
                        nxt_x = emit_x_dma(b + 1)
                    if hp + 1 < H // 2:
                        qk_t = emit_qk(hp + 1, xT_b)
                    srs = rcpp.tile([2, NP], F32, tag="srs")
                    srs_l.append(srs)
                    if hp >= 1:
                        emit_norm(hp - 1, srs_l[hp - 1], oc_sb)
                    for hi in (0, 1):
                        emit_av(hp, hi, es0, esg, v_tok, oc_sb, srs)
                prev_oc = oc_sb
                prev_srs = srs_l[H // 2 - 1]
                if b + 1 < BPC:
                    xT_b = nxt_x
                    v_tok = emit_v(xT_b)
                emit_norm(H // 2 - 1, prev_srs, prev_oc)
                emit_proj(b, prev_oc)

    nc.finalize()
    return nc


_CACHE = {}


def _make_in_maps(x, qkv_w, qkv_b, proj_w, proj_b):
    bf = ml_dtypes.bfloat16
    x = np.asarray(x, np.float32)
    qkv_w = np.asarray(qkv_w, np.float32)
    qkv_b = np.asarray(qkv_b, np.float32)
    proj_w = np.asarray(proj_w, np.float32)
    proj_b = np.asarray(proj_b, np.float32)

    wqkT = np.ascontiguousarray(qkv_w[: 2 * C].T).copy()
    wqkT[:, :C] *= SCALE
    wqkT = wqkT.astype(bf)
    wvT = np.ascontiguousarray(qkv_w[2 * C :].T).astype(bf)
    wpT = np.ascontiguousarray(proj_w.T).astype(bf)
    bqk_h = qkv_b[: 2 * C].copy()
    bqk_h[:C] *= SCALE
    bv_h = np.ascontiguousarray(qkv_b[2 * C :])

    m = np.zeros((NP, NP), np.float32)
    m[:N, :N] = _build_mask_np()
    mask0 = m[:P, :].astype(bf)
    maskw = np.zeros((P, MW), np.float32)
    for grp, off, a, blen, k0, ksz in WIN:
        base = grp * GW[0] + off
        maskw[:ksz, base] = 1.0
        maskw[:ksz, base + 1] = m[k0 : k0 + ksz, 1]
        maskw[:ksz, base + 2 : base + 2 + blen] = m[k0 : k0 + ksz, a : a + blen]
    maskw = maskw.astype(bf)

    in_maps = []
    for c in range(NCORES):
        xp_c = np.zeros((BPC, NP, C), np.float32)
        xp_c[:, :N, :] = x[c * BPC : (c + 1) * BPC]
        xT_c = np.ascontiguousarray(xp_c.reshape(TP, C).T).astype(bf)
        in_maps.append(
            {
                "xT": xT_c,
                "wqkT": wqkT,
                "wvT": wvT,
                "wpT": wpT,
                "bqk": bqk_h,
                "bv": bv_h,
                "bp": proj_b,
                "mask0d": mask0,
                "maskwd": maskw,
                "ones12": np.ones(H, bf),
            }
        )
    return in_maps


def kernel(x, qkv_w, qkv_b, proj_w, proj_b):
    if "nc" not in _CACHE:
        _CACHE["nc"] = _build_program()
    nc = _CACHE["nc"]

    in_maps = _make_in_maps(x, qkv_w, qkv_b, proj_w, proj_b)
    res = run_bass_kernel_spmd(nc, in_maps, list(range(NCORES)))
    out = np.concatenate(
        [res.results[c]["y"].reshape(BPC, N, C) for c in range(NCORES)], axis=0
    )
    return out.astype(np.float32)
